# revision 8
# baseline (speedup 1.0000x reference)
"""CRF negative-log-likelihood loss on 8 Trainium2 NeuronCores.

Strategy (data-parallel over batch, per the sharding hint):
  * Forward (partition-function) scan runs on-device in the exp domain:
    state p[label, batch_col] with a constant per-step normalizer C
    subtracted inside the emission exponential, so no renormalization or
    running log-Z tensor is needed (drift stays within fp32/bf16 range).
    Per step: one TensorE matmul q = exp(trans^T) @ p accumulating in
    PSUM, one VectorE multiply p = q * exp(emit - C).
  * Batch rows are sorted by descending sequence length (host side) and
    grouped in 8s (one row per core per group), each group processed for
    its max length; per-step live-column extents shrink so finished
    columns freeze in place. Rows shorter than their group max get their
    forward score captured by tiny boundary matmuls into PSUM slots.
  * Gold-path score and the final log/mean are tiny-tensor work done on
    the host; emissions (the only large tensor) stream through the
    device exactly once as bf16.
"""

import math
import os
import sys
import types

import numpy as np
import ml_dtypes

import concourse.bass as bass
import concourse.mybir as mybir
import concourse.tile as tile
from concourse.tile import TileContext
from concourse.bass_utils import run_bass_kernel_spmd
from bass_rust import ScopedClock

BF16 = ml_dtypes.bfloat16
B, S, L = 1024, 512, 64
NCORES = 8
BC = B // NCORES  # batch columns per core (= groups)
C_NORM = math.log(L) + 0.5  # per-step normalizer subtracted inside exp


# --- walrus in this image rejects >2 sem waits on one instruction; split the
# --- TileContext tail-drain waits into individual wait ops instead.
def _patched_drain_and_barrier(self, tick_clock, wait_clock):
    nc = self.nc
    probe = nc.sync.nop()
    wait_clock.add_sem_waits(probe.ins, ScopedClock({None: tick_clock.global_clock}))
    waits = list(probe.ins.sync_info.on_wait or [])
    if probe.ins.sync_info is not None:
        probe.ins.sync_info.on_wait = []
    by_name = {h.name: h for h in self.sems.allocated().values()}
    for w in waits:
        nc.sync.wait_ge(by_name[w.ant_name], w.wait_value)
    nc.sync.drain()
    nc.all_engine_barrier()
    popped = nc._tile_sem_poison_stack.pop()
    assert popped is self._sem_poison
    nc.clear_and_free_semaphores(list(self.sems.allocated().values()))
    nc.all_engine_barrier()


tile.TileContext._drain_and_barrier = _patched_drain_and_barrier


_MAXW = 1  # max sem waits walrus accepts alongside an update


def _split_excess_waits(nc):
    """Hoist waits beyond _MAXW onto preceding same-engine NOPs."""
    ctr = [0]
    for f in nc.m.functions:
        for blk in f.blocks:
            out = []
            for inst in blk.instructions:
                si = inst.sync_info
                if si is not None and si.on_wait and len(si.on_wait) > _MAXW:
                    waits = list(si.on_wait)
                    keep = waits[:_MAXW]
                    extra = waits[_MAXW:]
                    for i in range(0, len(extra), _MAXW):
                        ctr[0] += 1
                        out.append(mybir.InstNoOp(
                            name=f"I-wsplit-{ctr[0]}",
                            engine=inst.engine,
                            ins=[], outs=[],
                            sync_info=mybir.SyncInfo(
                                on_wait=extra[i:i + _MAXW], on_update=[]),
                        ))
                    si.on_wait = keep
                out.append(inst)
            blk.instructions[:] = out


def _round16(x):
    return (x + 15) // 16 * 16


def _build_schedule(lens_sorted):
    """lens_sorted: [B] descending. Returns dict with per-step extents,
    chunk extents, and capture records."""
    gmax = lens_sorted[0::8].astype(np.int64)  # [BC] group max len
    gmin = lens_sorted[7::8].astype(np.int64)  # [BC] group min len
    tmax = int(gmax[0])
    # live groups at step t (process step t iff n_t > 0); t=0 is init only
    n = np.array([int((gmax > t).sum()) for t in range(tmax)], dtype=np.int64)
    # chunk k covers steps 2k, 2k+1 ; chunk extent = n[2k]
    nchunks = (tmax + 1) // 2
    m = [int(n[2 * k]) for k in range(nchunks)]
    m16 = [_round16(v) for v in m]
    # capture records: group j captured at steps t in [gmin_j-1, gmax_j-2]
    cap = {}  # t -> (lo, hi)
    for j in range(BC):
        if gmin[j] >= gmax[j]:
            continue
        for t in range(int(gmin[j]) - 1, int(gmax[j]) - 1):
            lo, hi = cap.get(t, (j, j + 1))
            cap[t] = (min(lo, j), max(hi, j + 1))
    cap_recs = []  # (t, lo, hi, slot_base)
    slots = 0
    for t in sorted(cap.keys()):
        lo, hi = cap[t]
        cap_recs.append((t, lo, hi, slots))
        slots += hi - lo
    return dict(gmax=gmax, tmax=tmax, n=n, m=m, m16=m16,
                cap_recs=cap_recs, cap_slots=slots)


_PROGRAM_CACHE = {}


def _build_program(sched):
    """Build the per-core bass program for a given schedule."""
    dt = mybir.dt
    tmax, n, m16 = sched["tmax"], sched["n"], sched["m16"]
    nchunks = len(m16)
    rows_total = int(np.sum(m16))
    cap_recs, cap_slots = sched["cap_recs"], sched["cap_slots"]
    cap_total16 = max(16, _round16(cap_slots))
    cap_by_t = {t: (lo, hi, base) for (t, lo, hi, base) in cap_recs}

    nc = bass.Bass()
    # register -C as a usable activation-bias constant
    _ct = nc.alloc_sbuf_tensor(f"const-negC", [128, 1], dt.float32)
    nc.gpsimd.memset(_ct.ap(), -C_NORM)
    nc.const_aps.aps[(dt.float32, -C_NORM)] = _ct.ap()
    nc.all_engine_barrier()
    emp = nc.declare_dram_parameter("emp", [rows_total, 128], dt.bfloat16, isOutput=False)
    ematt = nc.declare_dram_parameter("ematt", [64, 64], dt.bfloat16, isOutput=False)
    startmc = nc.declare_dram_parameter("startmc", [64, 1], dt.float32, isOutput=False)
    eend = nc.declare_dram_parameter("eend", [64, 1], dt.bfloat16, isOutput=False)
    fnum_out = nc.declare_dram_parameter("fnum", [1, BC], dt.float32, isOutput=True)
    cap_out = nc.declare_dram_parameter("cap", [1, cap_total16], dt.float32, isOutput=True)

    row_off = np.concatenate([[0], np.cumsum(m16)]).astype(int)

    with TileContext(nc) as tc:
        with (
            tc.tile_pool(name="const", bufs=1) as cpool,
            tc.tile_pool(name="state", bufs=1) as spool,
            tc.tile_pool(name="em", bufs=4) as empool,
            tc.tile_pool(name="act", bufs=4) as apool,
            tc.tile_pool(name="psum", bufs=4, space="PSUM") as qpool,
            tc.tile_pool(name="psumcap", bufs=1, space="PSUM") as cappool,
            tc.tile_pool(name="psumf", bufs=1, space="PSUM") as fpool,
            tc.tile_pool(name="out", bufs=1) as opool,
        ):
            e_sb = cpool.tile([64, 64], dt.bfloat16, tag="e")
            nc.sync.dma_start(out=e_sb[:, :], in_=ematt[:, :])
            smc_sb = cpool.tile([64, 1], dt.float32, tag="smc")
            nc.sync.dma_start(out=smc_sb[:, :], in_=startmc[:, :])
            eend_sb = cpool.tile([64, 1], dt.bfloat16, tag="eend")
            nc.sync.dma_start(out=eend_sb[:, :], in_=eend[:, :])

            p = spool.tile([64, BC], dt.bfloat16, tag="p")
            cap_sb = opool.tile([1, cap_total16], dt.float32, tag="capsb")
            nc.vector.memset(cap_sb[0:1, :], 0.0)
            cap_ps = cappool.tile([1, 512], dt.float32, tag="capps")
            cap_used = 0   # slots used in current psum bank
            cap_done = 0   # slots drained to cap_sb

            def drain_caps():
                nonlocal cap_used, cap_done
                if cap_used:
                    nc.scalar.copy(out=cap_sb[0:1, cap_done:cap_done + cap_used],
                                   in_=cap_ps[0:1, 0:cap_used])
                    cap_done += cap_used
                    cap_used = 0

            def do_capture(t):
                nonlocal cap_used
                rec = cap_by_t.get(t)
                if rec is None:
                    return
                lo, hi, base = rec
                w = hi - lo
                if cap_used + w > 512:
                    drain_caps()
                nc.tensor.matmul(out=cap_ps[0:1, cap_used:cap_used + w],
                                 lhsT=eend_sb[:, 0:1], rhs=p[:, lo:hi],
                                 start=True, stop=True)
                cap_used += w

            a_tiles = {}
            em0 = None
            for k in range(nchunks):
                mk = m16[k]
                emt = empool.tile([128, 128], dt.bfloat16, tag="em")
                nc.sync.dma_start_transpose(
                    out=emt[:, 0:mk], in_=emp[row_off[k]:row_off[k] + mk, :])
                at = apool.tile([128, 128], dt.bfloat16, tag="a")
                if k == 0:
                    em0 = emt
                    # init p0 = exp(em0 + start - C) on labels of t=0
                    nc.scalar.activation(out=p[:, 0:BC], in_=emt[0:64, 0:BC],
                                         func=mybir.ActivationFunctionType.Exp,
                                         bias=smc_sb[:, 0:1], scale=1.0)
                    if tmax > 1:
                        nc.scalar.activation(out=at[64:128, 0:mk], in_=emt[64:128, 0:mk],
                                             func=mybir.ActivationFunctionType.Exp,
                                             bias=-C_NORM, scale=1.0)
                else:
                    nc.scalar.activation(out=at[:, 0:mk], in_=emt[:, 0:mk],
                                         func=mybir.ActivationFunctionType.Exp,
                                         bias=-C_NORM, scale=1.0)
                a_tiles[k] = at

            do_capture(0)  # captures of p0 (columns with true len 1 < groupmax)

            for t in range(1, tmax):
                nt = int(n[t])
                k, half = t // 2, t % 2
                at = a_tiles[k]
                q = qpool.tile([64, BC], dt.float32, tag="q")
                nc.tensor.matmul(out=q[:, 0:nt], lhsT=e_sb[:, :], rhs=p[:, 0:nt],
                                 start=True, stop=True)
                nc.vector.tensor_mul(out=p[:, 0:nt], in0=q[:, 0:nt],
                                     in1=at[64 * half:64 * half + 64, 0:nt])
                do_capture(t)

            drain_caps()
            fnum_ps = fpool.tile([1, BC], dt.float32, tag="fps")
            nc.tensor.matmul(out=fnum_ps[0:1, 0:BC], lhsT=eend_sb[:, 0:1],
                             rhs=p[:, 0:BC], start=True, stop=True)
            fnum_sb = opool.tile([1, BC], dt.float32, tag="fsb")
            nc.scalar.copy(out=fnum_sb[0:1, :], in_=fnum_ps[0:1, :])
            nc.sync.dma_start(out=fnum_out[:, :], in_=fnum_sb[0:1, :])
            nc.sync.dma_start(out=cap_out[:, :], in_=cap_sb[0:1, :])

    _split_excess_waits(nc)
    return nc


def _host_gold(emissions, tags, mask, transitions, start_t, end_t, lens):
    em64 = emissions.astype(np.float64)
    tr64 = transitions.astype(np.float64)
    emit_all = np.take_along_axis(em64, tags[:, :, None], axis=2)[..., 0]
    trans_all = tr64[tags[:, 1:], tags[:, :-1]]
    step = np.where(mask[:, 1:], trans_all + emit_all[:, 1:], 0.0)
    last = tags[np.arange(B), lens - 1]
    return (start_t.astype(np.float64)[tags[:, 0]] + emit_all[:, 0]
            + step.sum(1) + end_t.astype(np.float64)[last])


def _prepare(emissions, tags, mask, transitions, start_transitions,
             end_transitions):
    emissions = np.asarray(emissions, dtype=np.float32)
    tags = np.asarray(tags).astype(np.int64)
    mask = np.asarray(mask).astype(bool)
    transitions = np.asarray(transitions, dtype=np.float32)
    start_t = np.asarray(start_transitions, dtype=np.float32)
    end_t = np.asarray(end_transitions, dtype=np.float32)

    lens = mask.sum(1).astype(np.int64)
    order = np.argsort(-lens, kind="stable")
    sched = _build_schedule(lens[order])
    key = (sched["tmax"], tuple(sched["m16"]), tuple(sched["n"].tolist()),
           tuple(sched["cap_recs"]))
    if key not in _PROGRAM_CACHE:
        _PROGRAM_CACHE[key] = _build_program(sched)
    nc = _PROGRAM_CACHE[key]

    gmax, m16 = sched["gmax"], sched["m16"]
    nchunks = len(m16)
    ematt = np.exp(transitions.T).astype(BF16)        # [prev, next]
    startmc = (start_t - C_NORM).reshape(64, 1).astype(np.float32)
    eend = np.exp(end_t).reshape(64, 1).astype(BF16)
    in_maps = []
    core_orders = []
    for c in range(NCORES):
        oc = order[c::8]                               # this core's 128 rows, desc len
        core_orders.append(oc)
        emc = emissions[oc].astype(BF16)               # [BC, S, L]
        blocks = []
        for k in range(nchunks):
            mk_true = sched["m"][k]
            mk = m16[k]
            blk = emc[:mk_true, 2 * k:2 * k + 2, :].reshape(mk_true, 128)
            if mk > mk_true:
                pad = np.repeat(blk[-1:], mk - mk_true, axis=0) if mk_true else \
                    np.zeros((mk, 128), dtype=BF16)
                blk = np.concatenate([blk, pad], axis=0)
            blocks.append(blk)
        emp = np.ascontiguousarray(np.concatenate(blocks, axis=0))
        in_maps.append({"emp": emp, "ematt": ematt, "startmc": startmc,
                        "eend": eend})

    meta = dict(sched=sched, lens=lens, core_orders=core_orders,
                args=(emissions, tags, mask, transitions, start_t, end_t))
    return nc, in_maps, meta


def _postprocess(results, meta):
    sched, lens, core_orders = meta["sched"], meta["lens"], meta["core_orders"]
    emissions, tags, mask, transitions, start_t, end_t = meta["args"]
    gmax = sched["gmax"]
    cap_slot = {t: (lo, base) for (t, lo, hi, base) in sched["cap_recs"]}
    fwd = np.empty(B, dtype=np.float64)
    for c in range(NCORES):
        fnum = results[c]["fnum"][0].astype(np.float64)     # [BC]
        cap = results[c]["cap"][0].astype(np.float64)
        oc = core_orders[c]
        lc = lens[oc]
        for j in range(BC):
            Lj = int(lc[j])
            if Lj == int(gmax[j]):
                val = fnum[j]
            else:
                lo, base = cap_slot[Lj - 1]
                val = cap[base + (j - lo)]
            fwd[oc[j]] = math.log(val) + Lj * C_NORM
    gold = _host_gold(emissions, tags, mask, transitions, start_t, end_t, lens)
    return np.float32((fwd - gold).mean())


def kernel(emissions, tags, mask, transitions, start_transitions,
           end_transitions):
    nc, in_maps, meta = _prepare(emissions, tags, mask, transitions,
                                 start_transitions, end_transitions)
    res = run_bass_kernel_spmd(nc, in_maps, list(range(NCORES)))
    return _postprocess(res.results, meta)


# revision 9
# speedup vs baseline: 1.0023x; 1.0023x over previous
"""CRF negative-log-likelihood loss on 8 Trainium2 NeuronCores.

Strategy (data-parallel over batch, per the sharding hint):
  * Forward (partition-function) scan runs on-device in the exp domain:
    state p[label, batch_col] with a constant per-step normalizer C
    subtracted inside the emission exponential, so no renormalization or
    running log-Z tensor is needed (drift stays within fp32/bf16 range).
    Per step: one TensorE matmul q = exp(trans^T) @ p accumulating in
    PSUM, one VectorE multiply p = q * exp(emit - C).
  * Batch rows are sorted by descending sequence length (host side) and
    grouped in 8s (one row per core per group), each group processed for
    its max length; per-step live-column extents shrink so finished
    columns freeze in place. Rows shorter than their group max get their
    forward score captured by tiny boundary matmuls into PSUM slots.
  * Gold-path score and the final log/mean are tiny-tensor work done on
    the host; emissions (the only large tensor) stream through the
    device exactly once as bf16.
"""

import math
import os
import sys
import types

import numpy as np
import ml_dtypes

import concourse.bass as bass
import concourse.mybir as mybir
import concourse.tile as tile
from concourse.tile import TileContext
from concourse.bass_utils import run_bass_kernel_spmd
from bass_rust import ScopedClock

BF16 = ml_dtypes.bfloat16
B, S, L = 1024, 512, 64
NCORES = 8
BC = B // NCORES  # batch columns per core (= groups)
C_NORM = math.log(L) + 0.5  # per-step normalizer subtracted inside exp


# --- walrus in this image rejects >2 sem waits on one instruction; split the
# --- TileContext tail-drain waits into individual wait ops instead.
def _patched_drain_and_barrier(self, tick_clock, wait_clock):
    nc = self.nc
    probe = nc.sync.nop()
    wait_clock.add_sem_waits(probe.ins, ScopedClock({None: tick_clock.global_clock}))
    waits = list(probe.ins.sync_info.on_wait or [])
    if probe.ins.sync_info is not None:
        probe.ins.sync_info.on_wait = []
    by_name = {h.name: h for h in self.sems.allocated().values()}
    for w in waits:
        nc.sync.wait_ge(by_name[w.ant_name], w.wait_value)
    nc.sync.drain()
    nc.all_engine_barrier()
    popped = nc._tile_sem_poison_stack.pop()
    assert popped is self._sem_poison
    nc.clear_and_free_semaphores(list(self.sems.allocated().values()))
    nc.all_engine_barrier()


tile.TileContext._drain_and_barrier = _patched_drain_and_barrier


_MAXW = 1  # max sem waits walrus accepts alongside an update


def _split_excess_waits(nc):
    """Hoist waits beyond _MAXW onto preceding same-engine NOPs."""
    ctr = [0]
    for f in nc.m.functions:
        for blk in f.blocks:
            out = []
            for inst in blk.instructions:
                si = inst.sync_info
                if si is not None and si.on_wait and len(si.on_wait) > _MAXW:
                    waits = list(si.on_wait)
                    keep = waits[:_MAXW]
                    extra = waits[_MAXW:]
                    for i in range(0, len(extra), _MAXW):
                        ctr[0] += 1
                        out.append(mybir.InstNoOp(
                            name=f"I-wsplit-{ctr[0]}",
                            engine=inst.engine,
                            ins=[], outs=[],
                            sync_info=mybir.SyncInfo(
                                on_wait=extra[i:i + _MAXW], on_update=[]),
                        ))
                    si.on_wait = keep
                out.append(inst)
            blk.instructions[:] = out


def _round16(x):
    return (x + 15) // 16 * 16


def _build_schedule(lens_sorted):
    """lens_sorted: [B] descending. Returns dict with per-step extents,
    chunk extents, and capture records."""
    gmax = lens_sorted[0::8].astype(np.int64)  # [BC] group max len
    gmin = lens_sorted[7::8].astype(np.int64)  # [BC] group min len
    tmax = int(gmax[0])
    # live groups at step t (process step t iff n_t > 0); t=0 is init only
    n = np.array([int((gmax > t).sum()) for t in range(tmax)], dtype=np.int64)
    # chunk k covers steps 2k, 2k+1 ; chunk extent = n[2k]
    nchunks = (tmax + 1) // 2
    m = [int(n[2 * k]) for k in range(nchunks)]
    m16 = [_round16(v) for v in m]
    # capture records: group j captured at steps t in [gmin_j-1, gmax_j-2]
    cap = {}  # t -> (lo, hi)
    for j in range(BC):
        if gmin[j] >= gmax[j]:
            continue
        for t in range(int(gmin[j]) - 1, int(gmax[j]) - 1):
            lo, hi = cap.get(t, (j, j + 1))
            cap[t] = (min(lo, j), max(hi, j + 1))
    cap_recs = []  # (t, lo, hi, slot_base)
    slots = 0
    for t in sorted(cap.keys()):
        lo, hi = cap[t]
        cap_recs.append((t, lo, hi, slots))
        slots += hi - lo
    return dict(gmax=gmax, tmax=tmax, n=n, m=m, m16=m16,
                cap_recs=cap_recs, cap_slots=slots)


_PROGRAM_CACHE = {}


def _build_program(sched):
    """Build the per-core bass program for a given schedule."""
    dt = mybir.dt
    tmax, n, m16 = sched["tmax"], sched["n"], sched["m16"]
    nchunks = len(m16)
    rows_total = int(np.sum(m16))
    cap_recs, cap_slots = sched["cap_recs"], sched["cap_slots"]
    cap_total16 = max(16, _round16(cap_slots))
    cap_by_t = {t: (lo, hi, base) for (t, lo, hi, base) in cap_recs}

    nc = bass.Bass()
    # register -C as a usable activation-bias constant
    _ct = nc.alloc_sbuf_tensor(f"const-negC", [128, 1], dt.float32)
    nc.gpsimd.memset(_ct.ap(), -C_NORM)
    nc.const_aps.aps[(dt.float32, -C_NORM)] = _ct.ap()
    nc.all_engine_barrier()
    emp = nc.declare_dram_parameter("emp", [128, rows_total], dt.bfloat16, isOutput=False)
    ematt = nc.declare_dram_parameter("ematt", [64, 64], dt.bfloat16, isOutput=False)
    startmc = nc.declare_dram_parameter("startmc", [64, 1], dt.float32, isOutput=False)
    eend = nc.declare_dram_parameter("eend", [64, 1], dt.bfloat16, isOutput=False)
    fnum_out = nc.declare_dram_parameter("fnum", [1, BC], dt.float32, isOutput=True)
    cap_out = nc.declare_dram_parameter("cap", [1, cap_total16], dt.float32, isOutput=True)

    row_off = np.concatenate([[0], np.cumsum(m16)]).astype(int)

    with TileContext(nc) as tc:
        with (
            tc.tile_pool(name="const", bufs=1) as cpool,
            tc.tile_pool(name="state", bufs=1) as spool,
            tc.tile_pool(name="em", bufs=4) as empool,
            tc.tile_pool(name="act", bufs=4) as apool,
            tc.tile_pool(name="psum", bufs=4, space="PSUM") as qpool,
            tc.tile_pool(name="psumcap", bufs=1, space="PSUM") as cappool,
            tc.tile_pool(name="psumf", bufs=1, space="PSUM") as fpool,
            tc.tile_pool(name="out", bufs=1) as opool,
        ):
            e_sb = cpool.tile([64, 64], dt.bfloat16, tag="e")
            nc.sync.dma_start(out=e_sb[:, :], in_=ematt[:, :])
            smc_sb = cpool.tile([64, 1], dt.float32, tag="smc")
            nc.sync.dma_start(out=smc_sb[:, :], in_=startmc[:, :])
            eend_sb = cpool.tile([64, 1], dt.bfloat16, tag="eend")
            nc.sync.dma_start(out=eend_sb[:, :], in_=eend[:, :])

            p = spool.tile([64, BC], dt.bfloat16, tag="p")
            cap_sb = opool.tile([1, cap_total16], dt.float32, tag="capsb")
            nc.vector.memset(cap_sb[0:1, :], 0.0)
            cap_ps = cappool.tile([1, 512], dt.float32, tag="capps")
            cap_used = 0   # slots used in current psum bank
            cap_done = 0   # slots drained to cap_sb

            def drain_caps():
                nonlocal cap_used, cap_done
                if cap_used:
                    nc.scalar.copy(out=cap_sb[0:1, cap_done:cap_done + cap_used],
                                   in_=cap_ps[0:1, 0:cap_used])
                    cap_done += cap_used
                    cap_used = 0

            def do_capture(t):
                nonlocal cap_used
                rec = cap_by_t.get(t)
                if rec is None:
                    return
                lo, hi, base = rec
                w = hi - lo
                if cap_used + w > 512:
                    drain_caps()
                nc.tensor.matmul(out=cap_ps[0:1, cap_used:cap_used + w],
                                 lhsT=eend_sb[:, 0:1], rhs=p[:, lo:hi],
                                 start=True, stop=True)
                cap_used += w

            a_tiles = {}
            em0 = None
            for k in range(nchunks):
                mk = m16[k]
                emt = empool.tile([128, 128], dt.bfloat16, tag="em")
                nc.sync.dma_start(
                    out=emt[:, 0:mk], in_=emp[:, row_off[k]:row_off[k] + mk])
                at = apool.tile([128, 128], dt.bfloat16, tag="a")
                if k == 0:
                    em0 = emt
                    # init p0 = exp(em0 + start - C) on labels of t=0
                    nc.scalar.activation(out=p[:, 0:BC], in_=emt[0:64, 0:BC],
                                         func=mybir.ActivationFunctionType.Exp,
                                         bias=smc_sb[:, 0:1], scale=1.0)
                    if tmax > 1:
                        nc.scalar.activation(out=at[64:128, 0:mk], in_=emt[64:128, 0:mk],
                                             func=mybir.ActivationFunctionType.Exp,
                                             bias=-C_NORM, scale=1.0)
                else:
                    nc.scalar.activation(out=at[:, 0:mk], in_=emt[:, 0:mk],
                                         func=mybir.ActivationFunctionType.Exp,
                                         bias=-C_NORM, scale=1.0)
                a_tiles[k] = at

            do_capture(0)  # captures of p0 (columns with true len 1 < groupmax)

            for t in range(1, tmax):
                nt = int(n[t])
                k, half = t // 2, t % 2
                at = a_tiles[k]
                q = qpool.tile([64, BC], dt.float32, tag="q")
                nc.tensor.matmul(out=q[:, 0:nt], lhsT=e_sb[:, :], rhs=p[:, 0:nt],
                                 start=True, stop=True)
                nc.vector.tensor_mul(out=p[:, 0:nt], in0=q[:, 0:nt],
                                     in1=at[64 * half:64 * half + 64, 0:nt])
                do_capture(t)

            drain_caps()
            fnum_ps = fpool.tile([1, BC], dt.float32, tag="fps")
            nc.tensor.matmul(out=fnum_ps[0:1, 0:BC], lhsT=eend_sb[:, 0:1],
                             rhs=p[:, 0:BC], start=True, stop=True)
            fnum_sb = opool.tile([1, BC], dt.float32, tag="fsb")
            nc.scalar.copy(out=fnum_sb[0:1, :], in_=fnum_ps[0:1, :])
            nc.sync.dma_start(out=fnum_out[:, :], in_=fnum_sb[0:1, :])
            nc.sync.dma_start(out=cap_out[:, :], in_=cap_sb[0:1, :])

    _split_excess_waits(nc)
    return nc


def _host_gold(emissions, tags, mask, transitions, start_t, end_t, lens):
    em64 = emissions.astype(np.float64)
    tr64 = transitions.astype(np.float64)
    emit_all = np.take_along_axis(em64, tags[:, :, None], axis=2)[..., 0]
    trans_all = tr64[tags[:, 1:], tags[:, :-1]]
    step = np.where(mask[:, 1:], trans_all + emit_all[:, 1:], 0.0)
    last = tags[np.arange(B), lens - 1]
    return (start_t.astype(np.float64)[tags[:, 0]] + emit_all[:, 0]
            + step.sum(1) + end_t.astype(np.float64)[last])


def _prepare(emissions, tags, mask, transitions, start_transitions,
             end_transitions):
    emissions = np.asarray(emissions, dtype=np.float32)
    tags = np.asarray(tags).astype(np.int64)
    mask = np.asarray(mask).astype(bool)
    transitions = np.asarray(transitions, dtype=np.float32)
    start_t = np.asarray(start_transitions, dtype=np.float32)
    end_t = np.asarray(end_transitions, dtype=np.float32)

    lens = mask.sum(1).astype(np.int64)
    order = np.argsort(-lens, kind="stable")
    sched = _build_schedule(lens[order])
    key = (sched["tmax"], tuple(sched["m16"]), tuple(sched["n"].tolist()),
           tuple(sched["cap_recs"]))
    if key not in _PROGRAM_CACHE:
        _PROGRAM_CACHE[key] = _build_program(sched)
    nc = _PROGRAM_CACHE[key]

    gmax, m16 = sched["gmax"], sched["m16"]
    nchunks = len(m16)
    ematt = np.exp(transitions.T).astype(BF16)        # [prev, next]
    startmc = (start_t - C_NORM).reshape(64, 1).astype(np.float32)
    eend = np.exp(end_t).reshape(64, 1).astype(BF16)
    in_maps = []
    core_orders = []
    for c in range(NCORES):
        oc = order[c::8]                               # this core's 128 rows, desc len
        core_orders.append(oc)
        emc = emissions[oc].astype(BF16)               # [BC, S, L]
        blocks = []
        for k in range(nchunks):
            mk_true = sched["m"][k]
            mk = m16[k]
            blk = emc[:mk_true, 2 * k:2 * k + 2, :].reshape(mk_true, 128)
            if mk > mk_true:
                pad = np.repeat(blk[-1:], mk - mk_true, axis=0) if mk_true else \
                    np.zeros((mk, 128), dtype=BF16)
                blk = np.concatenate([blk, pad], axis=0)
            blocks.append(blk.T)
        emp = np.ascontiguousarray(np.concatenate(blocks, axis=1))
        in_maps.append({"emp": emp, "ematt": ematt, "startmc": startmc,
                        "eend": eend})

    meta = dict(sched=sched, lens=lens, core_orders=core_orders,
                args=(emissions, tags, mask, transitions, start_t, end_t))
    return nc, in_maps, meta


def _postprocess(results, meta):
    sched, lens, core_orders = meta["sched"], meta["lens"], meta["core_orders"]
    emissions, tags, mask, transitions, start_t, end_t = meta["args"]
    gmax = sched["gmax"]
    cap_slot = {t: (lo, base) for (t, lo, hi, base) in sched["cap_recs"]}
    fwd = np.empty(B, dtype=np.float64)
    for c in range(NCORES):
        fnum = results[c]["fnum"][0].astype(np.float64)     # [BC]
        cap = results[c]["cap"][0].astype(np.float64)
        oc = core_orders[c]
        lc = lens[oc]
        for j in range(BC):
            Lj = int(lc[j])
            if Lj == int(gmax[j]):
                val = fnum[j]
            else:
                lo, base = cap_slot[Lj - 1]
                val = cap[base + (j - lo)]
            fwd[oc[j]] = math.log(val) + Lj * C_NORM
    gold = _host_gold(emissions, tags, mask, transitions, start_t, end_t, lens)
    return np.float32((fwd - gold).mean())


def kernel(emissions, tags, mask, transitions, start_transitions,
           end_transitions):
    nc, in_maps, meta = _prepare(emissions, tags, mask, transitions,
                                 start_transitions, end_transitions)
    res = run_bass_kernel_spmd(nc, in_maps, list(range(NCORES)))
    return _postprocess(res.results, meta)
